# revision 19
# baseline (speedup 1.0000x reference)
"""CondensedLinearFineGrained on 8 TRN2 NeuronCores.

Math: out[b,o] = sum_k W[o,k] * input[b, mask[o,k]] + bias[o]
with B=256, IN_F=4096, OUT_F=4096, K=256.

Strategy
--------
Reformulate as a dense matmul:  out = input @ A^T + bias  where
A[o,f] = sum_{k: mask[o,k]==f} W[o,k]  (duplicates within a row are summed).

Sharding: output neurons, 512 per core. Per core:
  - input^T bf16 f-major tiles [128f x 32t x 256b] (2MB) via HWDGE.
  - A^T f-tiles [128f x 512o] bf16: the first N_DENSE_TILES arrive
    pre-densified from the host; the rest are built on-device by gpsimd
    local_scatter from host-packed CSC (idx+val packed in ONE tensor so a
    single early HWDGE DMA unblocks the scatters ~10us into the run).
  - TensorE accumulates psum[128b x 512o] over the 32 f-tiles; PSUM is
    seeded with bias via a K=1 bf16 matmul (ones^T @ bias).
  - Outputs ship as bf16 (rounds at ~2^-9 rel, well inside the 2e-2 gate)
    and are upcast on host.

v2 scheduling (vs the 37.4us baseline):
  - Feed chunks alternate across the two HWDGE queues in PE-consumption
    order; first chunks are small so PE starts ~9.5us instead of 14.3us.
  - CSC rides an early HWDGE slot (not SWDGE) so scatters start ~7us
    earlier and the scattered tiles act as PE filler work whenever the
    dense stream lags the PE's consumption rate.
  - The PE tile order is chosen by a small arrival-time simulation at
    build time: consume whichever tile (dense-in-arrival-order or next
    scattered) is available earliest.
  - Semaphores are cleared inline right after their last wait (the
    separate sem-recycle block cost ~1.4us), and the count is kept low
    (~21): the NEFF-exit quiesce costs ~220ns per semaphore.
  - DMAs that nobody waits on inc a junk semaphore (BIR requires an inc);
    per-engine FIFO ring order makes a later milestone DMA's completion
    imply theirs.
"""

import numpy as np
import ml_dtypes

B = 256
IN_F = 4096
OUT_F = 4096
K = 256
N_CORES = 8
O_SH = OUT_F // N_CORES  # 512 output rows per core
NT = IN_F // 128         # 32 feature tiles
NB = B // 128            # 2 batch tiles

N_DENSE_TILES = 16
N_JUNK = 21              # gap-free N=128 junk matmuls: keep the PE busy from
                         # block start until the first tile lands, so the HAM
                         # clock gate flips to 2.4GHz right as real work begins
N_JUNK_MID = 6           # first N real tiles also get 2 junk MMs after them:
                         # the early feed dribbles at ~700ns/tile, and these
                         # keep the HAM activity window busy through the gaps
CSC_SPLIT = 2            # pairs in the first (early) csc DMA

# dense tile groups: each group's in-chunk and atd-chunk go to OPPOSITE
# rings (swapped every group to balance bytes), so the rings deliver tiles
# in exact PE-consumption order at the aggregate HBM rate
DENSE_GROUPS = [(0, 1), (1, 2), (2, 4), (4, 6), (6, 8), (8, 10), (10, 13),
                (13, 16)]

_BF16 = ml_dtypes.bfloat16
_prog_cache = {}

# ---- arrival-time model constants (ns) for the build-time scheduler ----
_RATE = 0.30              # bytes/ns aggregate HBM rate (conservative)
_T_DATA0 = 1200           # block start -> first DMA byte moving
_T_LAG = 2000             # DMA wire-done -> completion sem visible
_T_SCAT = 1400            # per local_scatter pair
_MM_NS = 225              # warm matmul, N=512 (incl. wait slop)


def _plan(n_dense, wpad):
    """Static feed layout + PE tile order from a coarse arrival-time sim.

    Returns (qa, qb, in_chunks, atd_chunks, pe_order): qa/qb are lists of
    feed items ('bias'|'csc'|('in',c0,c1)|('atd',c0,c1)); pe_order is a
    list of (kind, idx) with kind 'd' (dense tile) or 's' (scattered).
    """
    assert n_dense == DENSE_GROUPS[-1][1]
    nt_s = NT - n_dense
    npair = nt_s // 2

    # scattered tiles' input chunks, interleaved into the dense stream
    sc_in = [(n_dense, n_dense + nt_s // 2), (n_dense + nt_s // 2, NT)]
    in_chunks = [g for g in DENSE_GROUPS] + sc_in
    atd_chunks = list(DENSE_GROUPS)

    in_bytes = lambda c0, c1: 128 * (c1 - c0) * B * 2
    atd_bytes = lambda c0, c1: 128 * (c1 - c0) * O_SH * 2
    csc_bytes = 128 * npair * 2 * wpad * 2

    cs = min(CSC_SPLIT, npair)
    csc1_bytes = 128 * cs * 2 * wpad * 2
    csc2_bytes = csc_bytes - csc1_bytes

    def nbytes(it):
        k, c0, c1 = it
        if k == 'bias':
            return 2 * O_SH
        if k == 'csc1':
            return csc1_bytes
        if k == 'csc2':
            return csc2_bytes
        return in_bytes(c0, c1) if k == 'in' else atd_bytes(c0, c1)

    # strict alternation: group g's in-chunk on one ring, atd-chunk on the
    # other, swapping each group. csc is split: the first CSC_SPLIT pairs
    # ride immediately after the first tile (so scatters start ~11us), the
    # rest after group 3. scattered-input chunks at ~1/2 and ~3/4; bias
    # second-to-last.
    qa, qb = [], []
    for g, (c0, c1) in enumerate(DENSE_GROUPS):
        (qa if g % 2 == 0 else qb).append(('in', c0, c1))
        (qb if g % 2 == 0 else qa).append(('atd', c0, c1))
        if g == 0:
            qa.append(('csc1', 0, 0))
        if g == 2 and csc2_bytes:
            qa.append(('csc2', 0, 0))
        if g == 3:
            qb.append(('in', *sc_in[0]))
        if g == 5:
            qa.append(('in', *sc_in[1]))
    qb.insert(len(qb) - 1, ('bias', 0, 0))

    # simulate both rings draining at half aggregate rate while both have
    # backlog, full rate once one is empty -> per-item completion times
    done = {}
    ra = [nbytes(it) for it in qa]
    rb = [nbytes(it) for it in qb]
    t = _T_DATA0
    pa = pb = 0
    offa = offb = 0.0
    while pa < len(qa) or pb < len(qb):
        both = pa < len(qa) and pb < len(qb)
        rate = _RATE / 2 if both else _RATE
        if pa < len(qa) and (not both or (ra[pa] - offa) <= (rb[pb] - offb)):
            dt = (ra[pa] - offa) / rate
            t += dt
            if both:
                offb += dt * rate
            done[id(qa[pa])] = t + _T_LAG
            offa = 0.0
            pa += 1
        else:
            dt = (rb[pb] - offb) / rate
            t += dt
            if both:
                offa += dt * rate
            done[id(qb[pb])] = t + _T_LAG
            offb = 0.0
            pb += 1

    t_item = {}
    for q in (qa, qb):
        for it in q:
            t_item[it] = done[id(it)]

    # per-tile availability
    in_done = {}
    for (c0, c1) in in_chunks:
        for tt in range(c0, c1):
            in_done[tt] = t_item[('in', c0, c1)]
    atd_done = {}
    for (c0, c1) in atd_chunks:
        for tt in range(c0, c1):
            atd_done[tt] = t_item[('atd', c0, c1)]
    t_csc1 = t_item[('csc1', 0, 0)]
    t_csc2 = t_item.get(('csc2', 0, 0), t_csc1)
    scat_done = {}
    t = t_csc1
    for j in range(npair):
        if j >= cs:
            t = max(t, t_csc2)
        t += _T_SCAT
        scat_done[j] = t

    avail = []
    for tt in range(n_dense):
        avail.append(('d', tt, max(in_done[tt], atd_done[tt])))
    for j in range(npair):
        for h in range(2):
            tt = n_dense + 2 * j + h
            avail.append(('s', 2 * j + h, max(in_done[tt], scat_done[j])))

    # greedy PE order: among not-yet-consumed tiles, dense in arrival
    # order and scattered in pair order, pick the earliest-available
    dense = sorted([a for a in avail if a[0] == 'd'], key=lambda a: a[2])
    scat = [a for a in avail if a[0] == 's']  # pair order
    order = []
    di = si = 0
    while di < len(dense) or si < len(scat):
        if di >= len(dense):
            order.append(scat[si][:2]); si += 1
        elif si >= len(scat):
            order.append(dense[di][:2]); di += 1
        elif dense[di][2] <= scat[si][2]:
            order.append(dense[di][:2]); di += 1
        else:
            order.append(scat[si][:2]); si += 1
    return qa, qb, in_chunks, atd_chunks, order


def _build_program(wpad: int, n_dense: int):
    """Hand-scheduled SPMD program: explicit per-engine streams + sems."""
    key = (wpad, n_dense)
    if key in _prog_cache:
        return _prog_cache[key]

    from contextlib import ExitStack
    from concourse import bacc, mybir, library_config

    nt_s = NT - n_dense
    npair = nt_s // 2
    assert nt_s % 2 == 0
    qa, qb, in_chunks, atd_ch, pe_order = _plan(n_dense, wpad)

    nc = bacc.Bacc("TRN2", target_bir_lowering=False, debug=False)
    dt = mybir.dt

    inT_d = nc.dram_tensor("inT", [128, NT, B], dt.bfloat16, kind="ExternalInput")
    bias_d = nc.dram_tensor("bias", [1, O_SH], dt.bfloat16, kind="ExternalInput")
    if npair:
        csc_d = nc.dram_tensor("csc", [128, npair, 2, wpad], dt.int16,
                               kind="ExternalInput")
    atd_d = nc.dram_tensor("atd", [128, n_dense, O_SH], dt.bfloat16,
                           kind="ExternalInput")
    out_d = nc.dram_tensor("out", [NB, 128, O_SH], dt.bfloat16,
                           kind="ExternalOutput")

    inT_sb = nc.alloc_sbuf_tensor("inT_sb", [128, NT, B], dt.bfloat16).ap()
    bias_sb = nc.alloc_sbuf_tensor("bias_sb", [1, O_SH], dt.bfloat16).ap()
    ones_sb = nc.alloc_sbuf_tensor("ones_sb", [1, 128], dt.bfloat16).ap()
    warm_sb = nc.alloc_sbuf_tensor("warm_sb", [128, 128], dt.bfloat16).ap()
    if npair:
        csc_sb = nc.alloc_sbuf_tensor("csc_sb", [128, npair, 2, wpad],
                                      dt.int16).ap()
        at_sb = nc.alloc_sbuf_tensor("at_sb", [128, npair, 2, O_SH],
                                     dt.bfloat16).ap()
    atd_sb = nc.alloc_sbuf_tensor("atd_sb", [128, n_dense, O_SH],
                                  dt.bfloat16).ap()
    outs_sb = [nc.alloc_sbuf_tensor(f"out_sb{i}", [128, O_SH], dt.bfloat16).ap()
               for i in range(NB)]

    psums = [nc.alloc_psum_tensor(f"ps{i}", [128, O_SH], dt.float32).ap()
             for i in range(NB)]
    ps_warm = nc.alloc_psum_tensor("ps_warm", [128, O_SH], dt.float32).ap()

    with ExitStack() as ctx:
        sem = lambda name: ctx.enter_context(nc.semaphore(name))
        s_junk = sem("s_junk")   # inc'd by unwaited DMAs; never waited/cleared
        s_bias = sem("s_bias")
        cs = min(CSC_SPLIT, npair)
        s_csc1 = sem("s_csc1") if npair else None
        s_csc2 = sem("s_csc2") if npair > cs else None
        s_in = {c: sem(f"s_in{c[0]}") for c in in_chunks}
        s_atd = {c: sem(f"s_atd{c[0]}") for c in atd_ch}
        s_g = sem("s_g")    # scatter pairs published
        s_v = sem("s_v")    # DVE consts ready
        s_ps = sem("s_ps")  # PE accumulation done per psum
        s_cp = [sem(f"s_cp{i}") for i in range(NB)]  # psum->sbuf copies

        def feed(eng, q):
            for it in q:
                kind, c0, c1 = it
                if kind == 'bias':
                    eng.dma_start(out=bias_sb[:], in_=bias_d[:]).then_inc(
                        s_bias, 16)
                elif kind == 'csc1':
                    eng.dma_start(out=csc_sb[:, :cs], in_=csc_d[:, :cs]
                                  ).then_inc(s_csc1, 16)
                elif kind == 'csc2':
                    eng.dma_start(out=csc_sb[:, cs:], in_=csc_d[:, cs:]
                                  ).then_inc(s_csc2, 16)
                elif kind == 'in':
                    eng.dma_start(out=inT_sb[:, c0:c1, :],
                                  in_=inT_d[:, c0:c1, :]).then_inc(
                                      s_in[(c0, c1)], 16)
                else:
                    eng.dma_start(out=atd_sb[:, c0:c1, :],
                                  in_=atd_d[:, c0:c1, :]).then_inc(
                                      s_atd[(c0, c1)], 16)

        with nc.Block() as block:

            @block.sync
            def _(sy):
                feed(sy, qa)
                sy.wait_ge(s_cp[0], 1)
                sy.sem_clear(s_cp[0])
                sy.dma_start(out=out_d[0], in_=outs_sb[0][:]).then_inc(
                    s_junk, 16)

            @block.scalar
            def _(sc):
                feed(sc, qb)
                sc.wait_ge(s_cp[1], 1)
                sc.sem_clear(s_cp[1])
                sc.dma_start(out=out_d[1], in_=outs_sb[1][:]).then_inc(
                    s_junk, 16)

            @block.vector
            def _(v):
                v.memset(ones_sb[:], 1.0)
                v.memset(warm_sb[:], 0.125)
                v.drain()
                v.sem_inc(s_v, 1)
                for i in range(NB):
                    v.wait_ge(s_ps, i + 1)
                    v.tensor_copy(outs_sb[i][:],
                                  psums[i][:]).then_inc(s_cp[i], 1)
                v.sem_clear(s_ps)

            if npair:
                @block.gpsimd
                def _(g):
                    g.load_library(library_config.local_scatter)
                    g.wait_ge(s_csc1, 16)
                    g.sem_clear(s_csc1)
                    for j in range(npair):
                        if j == cs and s_csc2 is not None:
                            g.wait_ge(s_csc2, 16)
                            g.sem_clear(s_csc2)
                        g.local_scatter(
                            at_sb[:, j],
                            csc_sb[:, j, 1, :],
                            csc_sb[:, j, 0, :],
                            channels=128,
                            num_elems=2 * O_SH,
                            num_idxs=wpad,
                        ).then_inc(s_g, 1)

            @block.tensor
            def _(te):
                # gap-free junk matmuls from block start until the first
                # tile is due: the HAM clock gate flips to 2.4GHz ~3.4us
                # after sustained PE activity begins, i.e. right as the
                # real stream starts, and never re-throttles after.
                te.wait_ge(s_v, 1)
                for _ in range(N_JUNK):
                    te.matmul(ps_warm[:, :128], warm_sb[:, :128],
                              warm_sb[:, :128],
                              start=True, stop=True, skip_group_check=True)
                te.sem_clear(s_v)

                in_seen = set()
                atd_seen = set()
                g_state = [0]

                def tile_rhs(kind, idx):
                    if kind == 'd':
                        t = idx
                        rhs = atd_sb[:, t, :]
                        for c in atd_ch:
                            if c[0] <= t < c[1] and c not in atd_seen:
                                te.wait_ge(s_atd[c], 16)
                                te.sem_clear(s_atd[c])
                                atd_seen.add(c)
                    else:
                        j, h = idx // 2, idx % 2
                        t = n_dense + idx
                        rhs = at_sb[:, j, h, :]
                        if j + 1 > g_state[0]:
                            te.wait_ge(s_g, j + 1)
                            g_state[0] = j + 1
                            if g_state[0] == npair:
                                te.sem_clear(s_g)
                    for c in in_chunks:
                        if c[0] <= t < c[1] and c not in in_seen:
                            te.wait_ge(s_in[c], 16)
                            te.sem_clear(s_in[c])
                            in_seen.add(c)
                    return t, rhs

                # stagger the two psums' completion: psum0 finishes (and
                # drains via DVE + DMA) while psum1's last matmuls run.
                # bias joins the accumulation LAST so its 1KB DMA stays off
                # the startup critical path.
                first = True
                tail_rhs = []
                for n, (kind, idx) in enumerate(pe_order):
                    t, rhs = tile_rhs(kind, idx)
                    if n >= len(pe_order) - 2:
                        tail_rhs.append((t, rhs))
                        te.matmul(psums[0][:], inT_sb[:, t, :128],
                                  rhs, start=False, stop=False)
                        continue
                    for i in range(NB):
                        te.matmul(psums[i][:],
                                  inT_sb[:, t, 128 * i:128 * (i + 1)],
                                  rhs, start=first, stop=False)
                    first = False
                    if n < N_JUNK_MID:
                        # early feed dribbles; keep the HAM activity window
                        # busy through the inter-tile gaps
                        for _ in range(2):
                            te.matmul(ps_warm[:, :128], warm_sb[:, :128],
                                      warm_sb[:, :128], start=True,
                                      stop=True, skip_group_check=True)

                te.wait_ge(s_bias, 16)
                te.sem_clear(s_bias)
                te.matmul(psums[0][:], ones_sb[:], bias_sb[:],
                          start=False, stop=True).then_inc(s_ps, 1)
                for t, rhs in tail_rhs:
                    te.matmul(psums[1][:], inT_sb[:, t, 128:256],
                              rhs, start=False, stop=False)
                te.matmul(psums[1][:], ones_sb[:], bias_sb[:],
                          start=False, stop=True).then_inc(s_ps, 1)

    nc.compile()
    _prog_cache[key] = nc
    return nc


def _prepare(input, condensed_weight, input_mask, bias):
    """Host-side repack: dedupe + CSC-bin the sparse weights, cast/transpose
    the activations. Returns (in_maps, wpad, n_dense)."""
    inT = np.ascontiguousarray(
        input.astype(_BF16).T.reshape(NT, 128, B).transpose(1, 0, 2))

    # dedupe (o, f) pairs, summing weights in f64
    o_idx = np.repeat(np.arange(OUT_F, dtype=np.int64), K)
    f_idx = input_mask.ravel().astype(np.int64)
    w = condensed_weight.ravel()
    key = (o_idx << 12) | f_idx
    uk, inv = np.unique(key, return_inverse=True)
    sums = np.bincount(inv, weights=w.astype(np.float64))
    o_u = (uk >> 12).astype(np.int64)
    f_u = (uk & (IN_F - 1)).astype(np.int64)
    v_u = sums.astype(np.float32)

    core = o_u // O_SH
    o_loc = o_u % O_SH
    t_id = f_u // 128
    p_f = f_u % 128

    n_dense = N_DENSE_TILES
    nt_s = NT - n_dense
    npair = nt_s // 2

    dense_m = t_id < n_dense
    atd = np.zeros((N_CORES, 128, n_dense, O_SH), dtype=_BF16)
    atd[core[dense_m], p_f[dense_m], t_id[dense_m], o_loc[dense_m]] = \
        v_u[dense_m]

    wpad = 2
    if npair:
        sm = ~dense_m
        ts = t_id[sm] - n_dense
        s_core, s_p, s_o, s_v = core[sm], p_f[sm], o_loc[sm], v_u[sm]
        s_pair = ts // 2
        s_idx = s_o + O_SH * (ts % 2)
        g = (s_core * 128 + s_p) * npair + s_pair
        order = np.argsort(g, kind="stable")
        gs = g[order]
        change = np.r_[True, gs[1:] != gs[:-1]]
        seg_start = np.flatnonzero(change)
        seg_id = np.cumsum(change) - 1
        rank = np.arange(gs.size) - seg_start[seg_id]

        maxc = int(rank.max()) + 1 if gs.size else 0
        wpad = max(2, (maxc + 1) // 2 * 2)

        # packed csc: [...,0,:] = int16 indices (-1 pad), [...,1,:] = bf16
        # value bits viewed as int16
        csc = np.zeros((N_CORES, 128, npair, 2, wpad), dtype=np.int16)
        csc[:, :, :, 0, :] = -1
        csc[s_core[order], s_p[order], s_pair[order], 0, rank] = \
            s_idx[order].astype(np.int16)
        vals = np.zeros((N_CORES, 128, npair, wpad), dtype=_BF16)
        vals[s_core[order], s_p[order], s_pair[order], rank] = s_v[order]
        csc[:, :, :, 1, :] = vals.view(np.int16)

    in_maps = []
    for c in range(N_CORES):
        m = {
            "inT": inT,
            "bias": np.ascontiguousarray(
                bias[c * O_SH:(c + 1) * O_SH].reshape(1, O_SH)
            ).astype(_BF16),
            "atd": np.ascontiguousarray(atd[c]),
        }
        if npair:
            m["csc"] = np.ascontiguousarray(csc[c])
        in_maps.append(m)
    return in_maps, wpad, n_dense


def kernel(input, condensed_weight, input_mask, bias,
           _run_kwargs=None, _res_box=None):
    """Full inputs in, full output out. Shards over 8 NeuronCores inside."""
    from concourse.bass_utils import run_bass_kernel_spmd

    in_maps, wpad, n_dense = _prepare(
        np.asarray(input), np.asarray(condensed_weight),
        np.asarray(input_mask), np.asarray(bias))
    nc = _build_program(wpad, n_dense)

    res = run_bass_kernel_spmd(nc, in_maps, list(range(N_CORES)),
                               **(_run_kwargs or {}))
    if _res_box is not None:
        _res_box["results"] = res

    out = np.concatenate(
        [np.asarray(res.results[c]["out"]).reshape(B, O_SH).astype(np.float32)
         for c in range(N_CORES)], axis=1)
    return out


# revision 24
# speedup vs baseline: 1.0602x; 1.0602x over previous
"""CondensedLinearFineGrained on 8 TRN2 NeuronCores.

Math: out[b,o] = sum_k W[o,k] * input[b, mask[o,k]] + bias[o]
with B=256, IN_F=4096, OUT_F=4096, K=256.

Strategy
--------
Reformulate as a dense matmul:  out = input @ A^T + bias  where
A[o,f] = sum_{k: mask[o,k]==f} W[o,k]  (duplicates within a row are summed).

Sharding: output neurons, 512 per core. Per core:
  - input^T bf16 f-major tiles [128f x 32t x 256b] (2MB) via HWDGE.
  - A^T f-tiles [128f x 512o] bf16: the first N_DENSE_TILES arrive
    pre-densified from the host; the rest are built on-device by gpsimd
    local_scatter from host-packed CSC (idx+val packed in ONE tensor so a
    single early HWDGE DMA unblocks the scatters ~10us into the run).
  - TensorE accumulates psum[128b x 512o] over the 32 f-tiles; PSUM is
    seeded with bias via a K=1 bf16 matmul (ones^T @ bias).
  - Outputs ship as bf16 (rounds at ~2^-9 rel, well inside the 2e-2 gate)
    and are upcast on host.

v2 scheduling (vs the 37.4us baseline):
  - Feed chunks alternate across the two HWDGE queues in PE-consumption
    order; first chunks are small so PE starts ~9.5us instead of 14.3us.
  - CSC rides an early HWDGE slot (not SWDGE) so scatters start ~7us
    earlier and the scattered tiles act as PE filler work whenever the
    dense stream lags the PE's consumption rate.
  - The PE tile order is chosen by a small arrival-time simulation at
    build time: consume whichever tile (dense-in-arrival-order or next
    scattered) is available earliest.
  - Semaphores are cleared inline right after their last wait (the
    separate sem-recycle block cost ~1.4us), and the count is kept low
    (~21): the NEFF-exit quiesce costs ~220ns per semaphore.
  - DMAs that nobody waits on inc a junk semaphore (BIR requires an inc);
    per-engine FIFO ring order makes a later milestone DMA's completion
    imply theirs.
"""

import numpy as np
import ml_dtypes

B = 256
IN_F = 4096
OUT_F = 4096
K = 256
N_CORES = 8
O_SH = OUT_F // N_CORES  # 512 output rows per core
NT = IN_F // 128         # 32 feature tiles
NB = B // 128            # 2 batch tiles

N_DENSE_TILES = 20
N_JUNK = 20              # gap-free N=128 junk matmuls: keep the PE busy from
                         # block start until the first tile lands, so the HAM
                         # clock gate flips to 2.4GHz right as real work begins
N_JUNK_MID = 8           # first N real tiles also get 2 junk MMs after them:
                         # the early feed dribbles at ~700ns/tile, and these
                         # keep the HAM activity window busy through the gaps
CSC_SPLIT = 2            # pairs in the first (early) csc DMA

# dense tile groups: each group's in-chunk and atd-chunk go to OPPOSITE
# rings (swapped every group to balance bytes), so the rings deliver tiles
# in exact PE-consumption order at the aggregate HBM rate
DENSE_GROUPS = [(0, 1), (1, 2), (2, 4), (4, 6), (6, 8), (8, 10), (10, 12),
                (12, 14), (14, 16), (16, 18), (18, 20)]

_BF16 = ml_dtypes.bfloat16
_prog_cache = {}

# ---- arrival-time model constants (ns) for the build-time scheduler ----
_RATE = 0.29              # bytes/ns aggregate HBM rate (conservative)
_T_DATA0 = 1200           # block start -> first DMA byte moving
_T_LAG = 2400             # DMA wire-done -> completion sem visible
_T_FLOOR = 3100           # earliest any item is visible, after block start
_T_SCAT = 1400            # per local_scatter pair
_S_MARGIN = 1200          # extra pessimism on scatter-tile availability
_MM_NS = 225              # warm matmul, N=512 (incl. wait slop)


def _plan(n_dense, wpad):
    """Static feed layout + PE tile order from a coarse arrival-time sim.

    Returns (qa, qb, in_chunks, atd_chunks, pe_order): qa/qb are lists of
    feed items ('bias'|'csc'|('in',c0,c1)|('atd',c0,c1)); pe_order is a
    list of (kind, idx) with kind 'd' (dense tile) or 's' (scattered).
    """
    assert n_dense == DENSE_GROUPS[-1][1]
    nt_s = NT - n_dense
    npair = nt_s // 2

    # scattered tiles' input chunks, interleaved into the dense stream
    sc_in = [(n_dense, n_dense + nt_s // 2), (n_dense + nt_s // 2, NT)]
    in_chunks = [g for g in DENSE_GROUPS] + sc_in
    atd_chunks = list(DENSE_GROUPS)

    in_bytes = lambda c0, c1: 128 * (c1 - c0) * B * 2
    atd_bytes = lambda c0, c1: 128 * (c1 - c0) * O_SH * 2
    csc_bytes = 128 * npair * 2 * wpad * 2

    cs = min(CSC_SPLIT, npair)
    csc1_bytes = 128 * cs * 2 * wpad * 2
    csc2_bytes = csc_bytes - csc1_bytes

    def nbytes(it):
        k, c0, c1 = it
        if k == 'bias':
            return 2 * O_SH
        if k == 'csc1':
            return csc1_bytes
        if k == 'csc2':
            return csc2_bytes
        return in_bytes(c0, c1) if k == 'in' else atd_bytes(c0, c1)

    # strict alternation: group g's in-chunk on one ring, atd-chunk on the
    # other, swapping each group. csc is split: the first CSC_SPLIT pairs
    # ride immediately after the first tile (so scatters start ~11us), the
    # rest after group 3. scattered-input chunks at ~1/2 and ~3/4; bias
    # second-to-last.
    qa, qb = [], []
    for g, (c0, c1) in enumerate(DENSE_GROUPS):
        (qa if g % 2 == 0 else qb).append(('in', c0, c1))
        (qb if g % 2 == 0 else qa).append(('atd', c0, c1))
        if g == 0:
            qa.append(('csc1', 0, 0))
        if g == 1 and csc2_bytes:
            qa.append(('csc2', 0, 0))
        if g == 3:
            qb.append(('in', *sc_in[0]))
        if g == 7:
            qa.append(('in', *sc_in[1]))
    qb.insert(len(qb) - 1, ('bias', 0, 0))

    # simulate both rings draining at half aggregate rate while both have
    # backlog, full rate once one is empty -> per-item completion times
    done = {}
    ra = [nbytes(it) for it in qa]
    rb = [nbytes(it) for it in qb]
    t = _T_DATA0
    pa = pb = 0
    offa = offb = 0.0
    while pa < len(qa) or pb < len(qb):
        both = pa < len(qa) and pb < len(qb)
        rate = _RATE / 2 if both else _RATE
        if pa < len(qa) and (not both or (ra[pa] - offa) <= (rb[pb] - offb)):
            dt = (ra[pa] - offa) / rate
            t += dt
            if both:
                offb += dt * rate
            done[id(qa[pa])] = max(t + _T_LAG, _T_FLOOR)
            offa = 0.0
            pa += 1
        else:
            dt = (rb[pb] - offb) / rate
            t += dt
            if both:
                offa += dt * rate
            done[id(qb[pb])] = max(t + _T_LAG, _T_FLOOR)
            offb = 0.0
            pb += 1

    t_item = {}
    for q in (qa, qb):
        for it in q:
            t_item[it] = done[id(it)]

    # per-tile availability
    in_done = {}
    for (c0, c1) in in_chunks:
        for tt in range(c0, c1):
            in_done[tt] = t_item[('in', c0, c1)]
    atd_done = {}
    for (c0, c1) in atd_chunks:
        for tt in range(c0, c1):
            atd_done[tt] = t_item[('atd', c0, c1)]
    t_csc1 = t_item[('csc1', 0, 0)]
    t_csc2 = t_item.get(('csc2', 0, 0), t_csc1)
    scat_done = {}
    t = t_csc1
    for j in range(npair):
        if j >= cs:
            t = max(t, t_csc2)
        t += _T_SCAT
        scat_done[j] = t

    avail = []
    for tt in range(n_dense):
        avail.append(('d', tt, max(in_done[tt], atd_done[tt])))
    for j in range(npair):
        for h in range(2):
            tt = n_dense + 2 * j + h
            avail.append(('s', 2 * j + h,
                          max(in_done[tt], scat_done[j] + _S_MARGIN)))

    # greedy PE order: among not-yet-consumed tiles, dense in arrival
    # order and scattered in pair order, pick the earliest-available
    dense = sorted([a for a in avail if a[0] == 'd'], key=lambda a: a[2])
    scat = [a for a in avail if a[0] == 's']  # pair order
    order = []
    di = si = 0
    while di < len(dense) or si < len(scat):
        if di >= len(dense):
            order.append(scat[si][:2]); si += 1
        elif si >= len(scat):
            order.append(dense[di][:2]); di += 1
        elif dense[di][2] <= scat[si][2]:
            order.append(dense[di][:2]); di += 1
        else:
            order.append(scat[si][:2]); si += 1
    return qa, qb, in_chunks, atd_chunks, order


def _build_program(wpad: int, n_dense: int):
    """Hand-scheduled SPMD program: explicit per-engine streams + sems."""
    key = (wpad, n_dense)
    if key in _prog_cache:
        return _prog_cache[key]

    from contextlib import ExitStack
    from concourse import bacc, mybir, library_config

    nt_s = NT - n_dense
    npair = nt_s // 2
    assert nt_s % 2 == 0
    qa, qb, in_chunks, atd_ch, pe_order = _plan(n_dense, wpad)

    nc = bacc.Bacc("TRN2", target_bir_lowering=False, debug=False)
    dt = mybir.dt

    inT_d = nc.dram_tensor("inT", [128, NT, B], dt.bfloat16, kind="ExternalInput")
    bias_d = nc.dram_tensor("bias", [1, O_SH], dt.bfloat16, kind="ExternalInput")
    if npair:
        csc_d = nc.dram_tensor("csc", [128, npair, 2, wpad], dt.int16,
                               kind="ExternalInput")
    atd_d = nc.dram_tensor("atd", [128, n_dense, O_SH], dt.bfloat16,
                           kind="ExternalInput")
    out_d = nc.dram_tensor("out", [NB, 128, O_SH], dt.bfloat16,
                           kind="ExternalOutput")

    inT_sb = nc.alloc_sbuf_tensor("inT_sb", [128, NT, B], dt.bfloat16).ap()
    bias_sb = nc.alloc_sbuf_tensor("bias_sb", [1, O_SH], dt.bfloat16).ap()
    ones_sb = nc.alloc_sbuf_tensor("ones_sb", [1, 128], dt.bfloat16).ap()
    warm_sb = nc.alloc_sbuf_tensor("warm_sb", [128, 128], dt.bfloat16).ap()
    if npair:
        csc_sb = nc.alloc_sbuf_tensor("csc_sb", [128, npair, 2, wpad],
                                      dt.int16).ap()
        at_sb = nc.alloc_sbuf_tensor("at_sb", [128, npair, 2, O_SH],
                                     dt.bfloat16).ap()
    atd_sb = nc.alloc_sbuf_tensor("atd_sb", [128, n_dense, O_SH],
                                  dt.bfloat16).ap()
    outs_sb = [nc.alloc_sbuf_tensor(f"out_sb{i}", [128, O_SH], dt.bfloat16).ap()
               for i in range(NB)]

    psums = [nc.alloc_psum_tensor(f"ps{i}", [128, O_SH], dt.float32).ap()
             for i in range(NB)]
    ps_warm = nc.alloc_psum_tensor("ps_warm", [128, O_SH], dt.float32).ap()

    with ExitStack() as ctx:
        sem = lambda name: ctx.enter_context(nc.semaphore(name))
        s_junk = sem("s_junk")   # inc'd by unwaited DMAs; never waited/cleared
        s_bias = sem("s_bias")
        cs = min(CSC_SPLIT, npair)
        s_csc1 = sem("s_csc1") if npair else None
        s_csc2 = sem("s_csc2") if npair > cs else None
        s_in = {c: sem(f"s_in{c[0]}") for c in in_chunks}
        s_atd = {c: sem(f"s_atd{c[0]}") for c in atd_ch}
        s_g = sem("s_g")    # scatter pairs published
        s_v = sem("s_v")    # DVE consts ready
        s_ps = sem("s_ps")  # PE accumulation done per psum
        s_cp = [sem(f"s_cp{i}") for i in range(NB)]  # psum->sbuf copies

        def feed(eng, q):
            for it in q:
                kind, c0, c1 = it
                if kind == 'bias':
                    eng.dma_start(out=bias_sb[:], in_=bias_d[:]).then_inc(
                        s_bias, 16)
                elif kind == 'csc1':
                    eng.dma_start(out=csc_sb[:, :cs], in_=csc_d[:, :cs]
                                  ).then_inc(s_csc1, 16)
                elif kind == 'csc2':
                    eng.dma_start(out=csc_sb[:, cs:], in_=csc_d[:, cs:]
                                  ).then_inc(s_csc2, 16)
                elif kind == 'in':
                    eng.dma_start(out=inT_sb[:, c0:c1, :],
                                  in_=inT_d[:, c0:c1, :]).then_inc(
                                      s_in[(c0, c1)], 16)
                else:
                    eng.dma_start(out=atd_sb[:, c0:c1, :],
                                  in_=atd_d[:, c0:c1, :]).then_inc(
                                      s_atd[(c0, c1)], 16)

        with nc.Block() as block:

            @block.sync
            def _(sy):
                feed(sy, qa)
                sy.wait_ge(s_cp[0], 1)
                sy.sem_clear(s_cp[0])
                sy.dma_start(out=out_d[0], in_=outs_sb[0][:]).then_inc(
                    s_junk, 16)

            @block.scalar
            def _(sc):
                feed(sc, qb)
                sc.wait_ge(s_cp[1], 1)
                sc.sem_clear(s_cp[1])
                sc.dma_start(out=out_d[1], in_=outs_sb[1][:]).then_inc(
                    s_junk, 16)

            @block.vector
            def _(v):
                v.memset(ones_sb[:], 1.0)
                v.memset(warm_sb[:], 0.125)
                v.drain()
                v.sem_inc(s_v, 1)
                for i in range(NB):
                    v.wait_ge(s_ps, i + 1)
                    v.tensor_copy(outs_sb[i][:],
                                  psums[i][:]).then_inc(s_cp[i], 1)
                v.sem_clear(s_ps)

            if npair:
                @block.gpsimd
                def _(g):
                    g.load_library(library_config.local_scatter)
                    g.wait_ge(s_csc1, 16)
                    g.sem_clear(s_csc1)
                    for j in range(npair):
                        if j == cs and s_csc2 is not None:
                            g.wait_ge(s_csc2, 16)
                            g.sem_clear(s_csc2)
                        g.local_scatter(
                            at_sb[:, j],
                            csc_sb[:, j, 1, :],
                            csc_sb[:, j, 0, :],
                            channels=128,
                            num_elems=2 * O_SH,
                            num_idxs=wpad,
                        ).then_inc(s_g, 1)

            @block.tensor
            def _(te):
                # gap-free junk matmuls from block start until the first
                # tile is due: the HAM clock gate flips to 2.4GHz ~3.4us
                # after sustained PE activity begins, i.e. right as the
                # real stream starts, and never re-throttles after.
                te.wait_ge(s_v, 1)
                for _ in range(N_JUNK):
                    te.matmul(ps_warm[:, :128], warm_sb[:, :128],
                              warm_sb[:, :128],
                              start=True, stop=True, skip_group_check=True)
                te.sem_clear(s_v)

                in_seen = set()
                atd_seen = set()
                g_state = [0]

                def tile_rhs(kind, idx):
                    if kind == 'd':
                        t = idx
                        rhs = atd_sb[:, t, :]
                        for c in atd_ch:
                            if c[0] <= t < c[1] and c not in atd_seen:
                                te.wait_ge(s_atd[c], 16)
                                te.sem_clear(s_atd[c])
                                atd_seen.add(c)
                    else:
                        j, h = idx // 2, idx % 2
                        t = n_dense + idx
                        rhs = at_sb[:, j, h, :]
                        if j + 1 > g_state[0]:
                            te.wait_ge(s_g, j + 1)
                            g_state[0] = j + 1
                            if g_state[0] == npair:
                                te.sem_clear(s_g)
                    for c in in_chunks:
                        if c[0] <= t < c[1] and c not in in_seen:
                            te.wait_ge(s_in[c], 16)
                            te.sem_clear(s_in[c])
                            in_seen.add(c)
                    return t, rhs

                # stagger the two psums' completion: psum0 finishes (and
                # drains via DVE + DMA) while psum1's last matmuls run.
                # bias joins the accumulation LAST so its 1KB DMA stays off
                # the startup critical path.
                first = True
                tail_rhs = []
                for n, (kind, idx) in enumerate(pe_order):
                    t, rhs = tile_rhs(kind, idx)
                    if n >= len(pe_order) - 2:
                        tail_rhs.append((t, rhs))
                        te.matmul(psums[0][:], inT_sb[:, t, :128],
                                  rhs, start=False, stop=False)
                        continue
                    for i in range(NB):
                        te.matmul(psums[i][:],
                                  inT_sb[:, t, 128 * i:128 * (i + 1)],
                                  rhs, start=first, stop=False)
                    first = False
                    if n < N_JUNK_MID:
                        # early feed dribbles; keep the HAM activity window
                        # busy through the inter-tile gaps
                        for _ in range(2):
                            te.matmul(ps_warm[:, :128], warm_sb[:, :128],
                                      warm_sb[:, :128], start=True,
                                      stop=True, skip_group_check=True)

                te.wait_ge(s_bias, 16)
                te.sem_clear(s_bias)
                te.matmul(psums[0][:], ones_sb[:], bias_sb[:],
                          start=False, stop=True).then_inc(s_ps, 1)
                for t, rhs in tail_rhs:
                    te.matmul(psums[1][:], inT_sb[:, t, 128:256],
                              rhs, start=False, stop=False)
                te.matmul(psums[1][:], ones_sb[:], bias_sb[:],
                          start=False, stop=True).then_inc(s_ps, 1)

    nc.compile()
    _prog_cache[key] = nc
    return nc


def _prepare(input, condensed_weight, input_mask, bias):
    """Host-side repack: dedupe + CSC-bin the sparse weights, cast/transpose
    the activations. Returns (in_maps, wpad, n_dense)."""
    inT = np.ascontiguousarray(
        input.astype(_BF16).T.reshape(NT, 128, B).transpose(1, 0, 2))

    # dedupe (o, f) pairs, summing weights in f64
    o_idx = np.repeat(np.arange(OUT_F, dtype=np.int64), K)
    f_idx = input_mask.ravel().astype(np.int64)
    w = condensed_weight.ravel()
    key = (o_idx << 12) | f_idx
    uk, inv = np.unique(key, return_inverse=True)
    sums = np.bincount(inv, weights=w.astype(np.float64))
    o_u = (uk >> 12).astype(np.int64)
    f_u = (uk & (IN_F - 1)).astype(np.int64)
    v_u = sums.astype(np.float32)

    core = o_u // O_SH
    o_loc = o_u % O_SH
    t_id = f_u // 128
    p_f = f_u % 128

    n_dense = N_DENSE_TILES
    nt_s = NT - n_dense
    npair = nt_s // 2

    dense_m = t_id < n_dense
    atd = np.zeros((N_CORES, 128, n_dense, O_SH), dtype=_BF16)
    atd[core[dense_m], p_f[dense_m], t_id[dense_m], o_loc[dense_m]] = \
        v_u[dense_m]

    wpad = 2
    if npair:
        sm = ~dense_m
        ts = t_id[sm] - n_dense
        s_core, s_p, s_o, s_v = core[sm], p_f[sm], o_loc[sm], v_u[sm]
        s_pair = ts // 2
        s_idx = s_o + O_SH * (ts % 2)
        g = (s_core * 128 + s_p) * npair + s_pair
        order = np.argsort(g, kind="stable")
        gs = g[order]
        change = np.r_[True, gs[1:] != gs[:-1]]
        seg_start = np.flatnonzero(change)
        seg_id = np.cumsum(change) - 1
        rank = np.arange(gs.size) - seg_start[seg_id]

        maxc = int(rank.max()) + 1 if gs.size else 0
        wpad = max(2, (maxc + 1) // 2 * 2)

        # packed csc: [...,0,:] = int16 indices (-1 pad), [...,1,:] = bf16
        # value bits viewed as int16
        csc = np.zeros((N_CORES, 128, npair, 2, wpad), dtype=np.int16)
        csc[:, :, :, 0, :] = -1
        csc[s_core[order], s_p[order], s_pair[order], 0, rank] = \
            s_idx[order].astype(np.int16)
        vals = np.zeros((N_CORES, 128, npair, wpad), dtype=_BF16)
        vals[s_core[order], s_p[order], s_pair[order], rank] = s_v[order]
        csc[:, :, :, 1, :] = vals.view(np.int16)

    in_maps = []
    for c in range(N_CORES):
        m = {
            "inT": inT,
            "bias": np.ascontiguousarray(
                bias[c * O_SH:(c + 1) * O_SH].reshape(1, O_SH)
            ).astype(_BF16),
            "atd": np.ascontiguousarray(atd[c]),
        }
        if npair:
            m["csc"] = np.ascontiguousarray(csc[c])
        in_maps.append(m)
    return in_maps, wpad, n_dense


def kernel(input, condensed_weight, input_mask, bias,
           _run_kwargs=None, _res_box=None):
    """Full inputs in, full output out. Shards over 8 NeuronCores inside."""
    from concourse.bass_utils import run_bass_kernel_spmd

    in_maps, wpad, n_dense = _prepare(
        np.asarray(input), np.asarray(condensed_weight),
        np.asarray(input_mask), np.asarray(bias))
    nc = _build_program(wpad, n_dense)

    res = run_bass_kernel_spmd(nc, in_maps, list(range(N_CORES)),
                               **(_run_kwargs or {}))
    if _res_box is not None:
        _res_box["results"] = res

    out = np.concatenate(
        [np.asarray(res.results[c]["out"]).reshape(B, O_SH).astype(np.float32)
         for c in range(N_CORES)], axis=1)
    return out


# revision 26
# speedup vs baseline: 1.1040x; 1.0413x over previous
"""CondensedLinearFineGrained on 8 TRN2 NeuronCores.

Math: out[b,o] = sum_k W[o,k] * input[b, mask[o,k]] + bias[o]
with B=256, IN_F=4096, OUT_F=4096, K=256.

Strategy
--------
Reformulate as a dense matmul:  out = input @ A^T + bias  where
A[o,f] = sum_{k: mask[o,k]==f} W[o,k]  (duplicates within a row are summed).

Sharding: output neurons, 512 per core. Per core:
  - input^T bf16 f-major tiles [128f x 32t x 256b] (2MB) via HWDGE.
  - A^T f-tiles [128f x 512o] bf16: the first N_DENSE_TILES arrive
    pre-densified from the host; the rest are built on-device by gpsimd
    local_scatter from host-packed CSC (idx+val packed in ONE tensor so a
    single early HWDGE DMA unblocks the scatters ~10us into the run).
  - TensorE accumulates psum[128b x 512o] over the 32 f-tiles; PSUM is
    seeded with bias via a K=1 bf16 matmul (ones^T @ bias).
  - Outputs ship as bf16 (rounds at ~2^-9 rel, well inside the 2e-2 gate)
    and are upcast on host.

v2 scheduling (vs the 37.4us baseline):
  - Feed chunks alternate across the two HWDGE queues in PE-consumption
    order; first chunks are small so PE starts ~9.5us instead of 14.3us.
  - CSC rides an early HWDGE slot (not SWDGE) so scatters start ~7us
    earlier and the scattered tiles act as PE filler work whenever the
    dense stream lags the PE's consumption rate.
  - The PE tile order is chosen by a small arrival-time simulation at
    build time: consume whichever tile (dense-in-arrival-order or next
    scattered) is available earliest.
  - Semaphores are cleared inline right after their last wait (the
    separate sem-recycle block cost ~1.4us), and the count is kept low
    (~21): the NEFF-exit quiesce costs ~220ns per semaphore.
  - DMAs that nobody waits on inc a junk semaphore (BIR requires an inc);
    per-engine FIFO ring order makes a later milestone DMA's completion
    imply theirs.
"""

import numpy as np
import ml_dtypes

B = 256
IN_F = 4096
OUT_F = 4096
K = 256
N_CORES = 8
O_SH = OUT_F // N_CORES  # 512 output rows per core
NT = IN_F // 128         # 32 feature tiles
NB = B // 128            # 2 batch tiles

N_DENSE_TILES = 20
N_JUNK = 20              # gap-free N=128 junk matmuls: keep the PE busy from
                         # block start until the first tile lands, so the HAM
                         # clock gate flips to 2.4GHz right as real work begins
N_JUNK_MID = 4           # first N real tiles also get 2 junk MMs after them:
                         # the early feed dribbles at ~700ns/tile, and these
                         # keep the HAM activity window busy through the gaps
CSC_SPLIT = 2            # pairs in the first (early) csc DMA

# dense tile groups: each group's in-chunk and atd-chunk go to OPPOSITE
# rings (swapped every group to balance bytes), so the rings deliver tiles
# in exact PE-consumption order at the aggregate HBM rate
DENSE_GROUPS = [(0, 1), (1, 2), (2, 4), (4, 6), (6, 8), (8, 10), (10, 12),
                (12, 14), (14, 16), (16, 18), (18, 20)]

_BF16 = ml_dtypes.bfloat16
_prog_cache = {}

# ---- arrival-time model constants (ns) for the build-time scheduler ----
_RATE = 0.29              # bytes/ns aggregate HBM rate (conservative)
_T_DATA0 = 1200           # block start -> first DMA byte moving
_T_LAG = 2400             # DMA wire-done -> completion sem visible
_T_FLOOR = 3100           # earliest any item is visible, after block start
_T_SCAT = 1400            # per local_scatter pair
_S_MARGIN = 1200          # extra pessimism on scatter-tile availability
_MM_NS = 225              # warm matmul, N=512 (incl. wait slop)


def _plan(n_dense, wpad):
    """Static feed layout + PE tile order from a coarse arrival-time sim.

    Returns (qa, qb, in_chunks, atd_chunks, pe_order): qa/qb are lists of
    feed items ('bias'|'csc'|('in',c0,c1)|('atd',c0,c1)); pe_order is a
    list of (kind, idx) with kind 'd' (dense tile) or 's' (scattered).
    """
    assert n_dense == DENSE_GROUPS[-1][1]
    nt_s = NT - n_dense
    npair = nt_s // 2

    # scattered tiles' input chunks, interleaved into the dense stream
    sc_in = [(n_dense, n_dense + nt_s // 2), (n_dense + nt_s // 2, NT)]
    in_chunks = [g for g in DENSE_GROUPS] + sc_in
    atd_chunks = list(DENSE_GROUPS)

    in_bytes = lambda c0, c1: 128 * (c1 - c0) * B * 2
    atd_bytes = lambda c0, c1: 128 * (c1 - c0) * O_SH * 2
    csc_bytes = 128 * npair * 2 * wpad * 2

    cs = min(CSC_SPLIT, npair)
    csc1_bytes = 128 * cs * 2 * wpad * 2
    csc2_bytes = csc_bytes - csc1_bytes

    def nbytes(it):
        k, c0, c1 = it
        if k == 'bias':
            return 2 * O_SH
        if k == 'csc1':
            return csc1_bytes
        if k == 'csc2':
            return csc2_bytes
        return in_bytes(c0, c1) if k == 'in' else atd_bytes(c0, c1)

    # strict alternation: group g's in-chunk on one ring, atd-chunk on the
    # other, swapping each group. csc is split: the first CSC_SPLIT pairs
    # ride immediately after the first tile (so scatters start ~11us), the
    # rest after group 3. scattered-input chunks at ~1/2 and ~3/4; bias
    # second-to-last.
    qa, qb = [], []
    for g, (c0, c1) in enumerate(DENSE_GROUPS):
        (qa if g % 2 == 0 else qb).append(('in', c0, c1))
        (qb if g % 2 == 0 else qa).append(('atd', c0, c1))
        if g == 0:
            qa.append(('csc1', 0, 0))
        if g == 4 and csc2_bytes:
            qb.append(('csc2', 0, 0))
        if g == 3:
            qb.append(('in', *sc_in[0]))
        if g == 7:
            qa.append(('in', *sc_in[1]))
    qb.insert(len(qb) - 1, ('bias', 0, 0))

    # simulate both rings draining at half aggregate rate while both have
    # backlog, full rate once one is empty -> per-item completion times
    done = {}
    ra = [nbytes(it) for it in qa]
    rb = [nbytes(it) for it in qb]
    t = _T_DATA0
    pa = pb = 0
    offa = offb = 0.0
    while pa < len(qa) or pb < len(qb):
        both = pa < len(qa) and pb < len(qb)
        rate = _RATE / 2 if both else _RATE
        if pa < len(qa) and (not both or (ra[pa] - offa) <= (rb[pb] - offb)):
            dt = (ra[pa] - offa) / rate
            t += dt
            if both:
                offb += dt * rate
            done[id(qa[pa])] = max(t + _T_LAG, _T_FLOOR)
            offa = 0.0
            pa += 1
        else:
            dt = (rb[pb] - offb) / rate
            t += dt
            if both:
                offa += dt * rate
            done[id(qb[pb])] = max(t + _T_LAG, _T_FLOOR)
            offb = 0.0
            pb += 1

    t_item = {}
    for q in (qa, qb):
        for it in q:
            t_item[it] = done[id(it)]

    # per-tile availability
    in_done = {}
    for (c0, c1) in in_chunks:
        for tt in range(c0, c1):
            in_done[tt] = t_item[('in', c0, c1)]
    atd_done = {}
    for (c0, c1) in atd_chunks:
        for tt in range(c0, c1):
            atd_done[tt] = t_item[('atd', c0, c1)]
    t_csc1 = t_item[('csc1', 0, 0)]
    t_csc2 = t_item.get(('csc2', 0, 0), t_csc1)
    scat_done = {}
    t = t_csc1
    for j in range(npair):
        if j >= cs:
            t = max(t, t_csc2)
        t += _T_SCAT
        scat_done[j] = t

    avail = []
    for tt in range(n_dense):
        avail.append(('d', tt, max(in_done[tt], atd_done[tt])))
    for j in range(npair):
        for h in range(2):
            tt = n_dense + 2 * j + h
            avail.append(('s', 2 * j + h,
                          max(in_done[tt], scat_done[j] + _S_MARGIN)))

    # greedy PE order: among not-yet-consumed tiles, dense in arrival
    # order and scattered in pair order, pick the earliest-available
    dense = sorted([a for a in avail if a[0] == 'd'], key=lambda a: a[2])
    scat = [a for a in avail if a[0] == 's']  # pair order
    order = []
    di = si = 0
    while di < len(dense) or si < len(scat):
        if di >= len(dense):
            order.append(scat[si][:2]); si += 1
        elif si >= len(scat):
            order.append(dense[di][:2]); di += 1
        elif dense[di][2] <= scat[si][2]:
            order.append(dense[di][:2]); di += 1
        else:
            order.append(scat[si][:2]); si += 1
    return qa, qb, in_chunks, atd_chunks, order


def _build_program(wpad: int, n_dense: int):
    """Hand-scheduled SPMD program: explicit per-engine streams + sems."""
    key = (wpad, n_dense)
    if key in _prog_cache:
        return _prog_cache[key]

    from contextlib import ExitStack
    from concourse import bacc, mybir, library_config

    nt_s = NT - n_dense
    npair = nt_s // 2
    assert nt_s % 2 == 0
    qa, qb, in_chunks, atd_ch, pe_order = _plan(n_dense, wpad)

    nc = bacc.Bacc("TRN2", target_bir_lowering=False, debug=False)
    dt = mybir.dt

    inT_d = nc.dram_tensor("inT", [128, NT, B], dt.bfloat16, kind="ExternalInput")
    bias_d = nc.dram_tensor("bias", [1, O_SH], dt.bfloat16, kind="ExternalInput")
    if npair:
        csc_d = nc.dram_tensor("csc", [128, npair, 2, wpad], dt.int16,
                               kind="ExternalInput")
    atd_d = nc.dram_tensor("atd", [128, n_dense, O_SH], dt.bfloat16,
                           kind="ExternalInput")
    out_d = nc.dram_tensor("out", [NB, 128, O_SH], dt.bfloat16,
                           kind="ExternalOutput")

    inT_sb = nc.alloc_sbuf_tensor("inT_sb", [128, NT, B], dt.bfloat16).ap()
    bias_sb = nc.alloc_sbuf_tensor("bias_sb", [1, O_SH], dt.bfloat16).ap()
    ones_sb = nc.alloc_sbuf_tensor("ones_sb", [1, 128], dt.bfloat16).ap()
    warm_sb = nc.alloc_sbuf_tensor("warm_sb", [128, 128], dt.bfloat16).ap()
    if npair:
        csc_sb = nc.alloc_sbuf_tensor("csc_sb", [128, npair, 2, wpad],
                                      dt.int16).ap()
        at_sb = nc.alloc_sbuf_tensor("at_sb", [128, npair, 2, O_SH],
                                     dt.bfloat16).ap()
    atd_sb = nc.alloc_sbuf_tensor("atd_sb", [128, n_dense, O_SH],
                                  dt.bfloat16).ap()
    outs_sb = [nc.alloc_sbuf_tensor(f"out_sb{i}", [128, O_SH], dt.bfloat16).ap()
               for i in range(NB)]

    psums = [nc.alloc_psum_tensor(f"ps{i}", [128, O_SH], dt.float32).ap()
             for i in range(NB)]
    ps_warm = nc.alloc_psum_tensor("ps_warm", [128, O_SH], dt.float32).ap()

    with ExitStack() as ctx:
        sem = lambda name: ctx.enter_context(nc.semaphore(name))
        s_junk = sem("s_junk")   # inc'd by unwaited DMAs; never waited/cleared
        s_bias = sem("s_bias")
        cs = min(CSC_SPLIT, npair)
        s_csc1 = sem("s_csc1") if npair else None
        s_csc2 = sem("s_csc2") if npair > cs else None
        s_in = {c: sem(f"s_in{c[0]}") for c in in_chunks}
        s_atd = {c: sem(f"s_atd{c[0]}") for c in atd_ch}
        s_g = sem("s_g")    # scatter pairs published
        s_v = sem("s_v")    # DVE consts ready
        s_ps = sem("s_ps")  # PE accumulation done per psum
        s_cp = [sem(f"s_cp{i}") for i in range(NB)]  # psum->sbuf copies

        def feed(eng, q):
            for it in q:
                kind, c0, c1 = it
                if kind == 'bias':
                    eng.dma_start(out=bias_sb[:], in_=bias_d[:]).then_inc(
                        s_bias, 16)
                elif kind == 'csc1':
                    eng.dma_start(out=csc_sb[:, :cs], in_=csc_d[:, :cs]
                                  ).then_inc(s_csc1, 16)
                elif kind == 'csc2':
                    eng.dma_start(out=csc_sb[:, cs:], in_=csc_d[:, cs:]
                                  ).then_inc(s_csc2, 16)
                elif kind == 'in':
                    eng.dma_start(out=inT_sb[:, c0:c1, :],
                                  in_=inT_d[:, c0:c1, :]).then_inc(
                                      s_in[(c0, c1)], 16)
                else:
                    eng.dma_start(out=atd_sb[:, c0:c1, :],
                                  in_=atd_d[:, c0:c1, :]).then_inc(
                                      s_atd[(c0, c1)], 16)

        with nc.Block() as block:

            @block.sync
            def _(sy):
                feed(sy, qa)
                sy.wait_ge(s_cp[0], 1)
                sy.sem_clear(s_cp[0])
                sy.dma_start(out=out_d[0], in_=outs_sb[0][:]).then_inc(
                    s_junk, 16)

            @block.scalar
            def _(sc):
                feed(sc, qb)
                sc.wait_ge(s_cp[1], 1)
                sc.sem_clear(s_cp[1])
                sc.dma_start(out=out_d[1], in_=outs_sb[1][:]).then_inc(
                    s_junk, 16)

            @block.vector
            def _(v):
                v.memset(ones_sb[:], 1.0)
                v.memset(warm_sb[:], 0.125)
                v.drain()
                v.sem_inc(s_v, 1)
                for i in range(NB):
                    v.wait_ge(s_ps, i + 1)
                    v.tensor_copy(outs_sb[i][:],
                                  psums[i][:]).then_inc(s_cp[i], 1)
                v.sem_clear(s_ps)

            if npair:
                @block.gpsimd
                def _(g):
                    g.load_library(library_config.local_scatter)
                    g.wait_ge(s_csc1, 16)
                    g.sem_clear(s_csc1)
                    for j in range(npair):
                        if j == cs and s_csc2 is not None:
                            g.wait_ge(s_csc2, 16)
                            g.sem_clear(s_csc2)
                        g.local_scatter(
                            at_sb[:, j],
                            csc_sb[:, j, 1, :],
                            csc_sb[:, j, 0, :],
                            channels=128,
                            num_elems=2 * O_SH,
                            num_idxs=wpad,
                        ).then_inc(s_g, 1)

            @block.tensor
            def _(te):
                # gap-free junk matmuls from block start until the first
                # tile is due: the HAM clock gate flips to 2.4GHz ~3.4us
                # after sustained PE activity begins, i.e. right as the
                # real stream starts, and never re-throttles after.
                te.wait_ge(s_v, 1)
                for _ in range(N_JUNK):
                    te.matmul(ps_warm[:, :128], warm_sb[:, :128],
                              warm_sb[:, :128],
                              start=True, stop=True, skip_group_check=True)
                te.sem_clear(s_v)

                in_seen = set()
                atd_seen = set()
                g_state = [0]

                def tile_rhs(kind, idx):
                    if kind == 'd':
                        t = idx
                        rhs = atd_sb[:, t, :]
                        for c in atd_ch:
                            if c[0] <= t < c[1] and c not in atd_seen:
                                te.wait_ge(s_atd[c], 16)
                                te.sem_clear(s_atd[c])
                                atd_seen.add(c)
                    else:
                        j, h = idx // 2, idx % 2
                        t = n_dense + idx
                        rhs = at_sb[:, j, h, :]
                        if j + 1 > g_state[0]:
                            te.wait_ge(s_g, j + 1)
                            g_state[0] = j + 1
                            if g_state[0] == npair:
                                te.sem_clear(s_g)
                    for c in in_chunks:
                        if c[0] <= t < c[1] and c not in in_seen:
                            te.wait_ge(s_in[c], 16)
                            te.sem_clear(s_in[c])
                            in_seen.add(c)
                    return t, rhs

                # stagger the two psums' completion: psum0 finishes (and
                # drains via DVE + DMA) while psum1's last matmuls run.
                # bias joins the accumulation LAST so its 1KB DMA stays off
                # the startup critical path.
                first = True
                tail_rhs = []
                for n, (kind, idx) in enumerate(pe_order):
                    t, rhs = tile_rhs(kind, idx)
                    if n >= len(pe_order) - 2:
                        tail_rhs.append((t, rhs))
                        te.matmul(psums[0][:], inT_sb[:, t, :128],
                                  rhs, start=False, stop=False)
                        continue
                    for i in range(NB):
                        te.matmul(psums[i][:],
                                  inT_sb[:, t, 128 * i:128 * (i + 1)],
                                  rhs, start=first, stop=False)
                    first = False
                    if n < N_JUNK_MID:
                        # early feed dribbles; keep the HAM activity window
                        # busy through the inter-tile gaps
                        for _ in range(2):
                            te.matmul(ps_warm[:, :128], warm_sb[:, :128],
                                      warm_sb[:, :128], start=True,
                                      stop=True, skip_group_check=True)

                te.wait_ge(s_bias, 16)
                te.sem_clear(s_bias)
                te.matmul(psums[0][:], ones_sb[:], bias_sb[:],
                          start=False, stop=True).then_inc(s_ps, 1)
                for t, rhs in tail_rhs:
                    te.matmul(psums[1][:], inT_sb[:, t, 128:256],
                              rhs, start=False, stop=False)
                te.matmul(psums[1][:], ones_sb[:], bias_sb[:],
                          start=False, stop=True).then_inc(s_ps, 1)

    nc.compile()
    _prog_cache[key] = nc
    return nc


def _prepare(input, condensed_weight, input_mask, bias):
    """Host-side repack: dedupe + CSC-bin the sparse weights, cast/transpose
    the activations. Returns (in_maps, wpad, n_dense)."""
    inT = np.ascontiguousarray(
        input.astype(_BF16).T.reshape(NT, 128, B).transpose(1, 0, 2))

    # dedupe (o, f) pairs, summing weights in f64
    o_idx = np.repeat(np.arange(OUT_F, dtype=np.int64), K)
    f_idx = input_mask.ravel().astype(np.int64)
    w = condensed_weight.ravel()
    key = (o_idx << 12) | f_idx
    uk, inv = np.unique(key, return_inverse=True)
    sums = np.bincount(inv, weights=w.astype(np.float64))
    o_u = (uk >> 12).astype(np.int64)
    f_u = (uk & (IN_F - 1)).astype(np.int64)
    v_u = sums.astype(np.float32)

    core = o_u // O_SH
    o_loc = o_u % O_SH
    t_id = f_u // 128
    p_f = f_u % 128

    n_dense = N_DENSE_TILES
    nt_s = NT - n_dense
    npair = nt_s // 2

    dense_m = t_id < n_dense
    atd = np.zeros((N_CORES, 128, n_dense, O_SH), dtype=_BF16)
    atd[core[dense_m], p_f[dense_m], t_id[dense_m], o_loc[dense_m]] = \
        v_u[dense_m]

    wpad = 2
    if npair:
        sm = ~dense_m
        ts = t_id[sm] - n_dense
        s_core, s_p, s_o, s_v = core[sm], p_f[sm], o_loc[sm], v_u[sm]
        s_pair = ts // 2
        s_idx = s_o + O_SH * (ts % 2)
        g = (s_core * 128 + s_p) * npair + s_pair
        order = np.argsort(g, kind="stable")
        gs = g[order]
        change = np.r_[True, gs[1:] != gs[:-1]]
        seg_start = np.flatnonzero(change)
        seg_id = np.cumsum(change) - 1
        rank = np.arange(gs.size) - seg_start[seg_id]

        maxc = int(rank.max()) + 1 if gs.size else 0
        wpad = max(2, (maxc + 1) // 2 * 2)

        # packed csc: [...,0,:] = int16 indices (-1 pad), [...,1,:] = bf16
        # value bits viewed as int16
        csc = np.zeros((N_CORES, 128, npair, 2, wpad), dtype=np.int16)
        csc[:, :, :, 0, :] = -1
        csc[s_core[order], s_p[order], s_pair[order], 0, rank] = \
            s_idx[order].astype(np.int16)
        vals = np.zeros((N_CORES, 128, npair, wpad), dtype=_BF16)
        vals[s_core[order], s_p[order], s_pair[order], rank] = s_v[order]
        csc[:, :, :, 1, :] = vals.view(np.int16)

    in_maps = []
    for c in range(N_CORES):
        m = {
            "inT": inT,
            "bias": np.ascontiguousarray(
                bias[c * O_SH:(c + 1) * O_SH].reshape(1, O_SH)
            ).astype(_BF16),
            "atd": np.ascontiguousarray(atd[c]),
        }
        if npair:
            m["csc"] = np.ascontiguousarray(csc[c])
        in_maps.append(m)
    return in_maps, wpad, n_dense


def kernel(input, condensed_weight, input_mask, bias,
           _run_kwargs=None, _res_box=None):
    """Full inputs in, full output out. Shards over 8 NeuronCores inside."""
    from concourse.bass_utils import run_bass_kernel_spmd

    in_maps, wpad, n_dense = _prepare(
        np.asarray(input), np.asarray(condensed_weight),
        np.asarray(input_mask), np.asarray(bias))
    nc = _build_program(wpad, n_dense)

    res = run_bass_kernel_spmd(nc, in_maps, list(range(N_CORES)),
                               **(_run_kwargs or {}))
    if _res_box is not None:
        _res_box["results"] = res

    out = np.concatenate(
        [np.asarray(res.results[c]["out"]).reshape(B, O_SH).astype(np.float32)
         for c in range(N_CORES)], axis=1)
    return out
